# revision 1
# baseline (speedup 1.0000x reference)
"""BertLayer on 8 trn2 NeuronCores — data-parallel over batch (2 per core).

v8 layout strategy (per core, tokens T=1024 = 2 batches x 512):
  - xT [hidden, tokens] is transposed on the HOST and DMA'd in directly
    (bf16); x natural (residual) is host-cast bf16.  Startup DMAs are
    interleaved across issuing engines so the first V matmul starts early.
  - V is produced natural with a ones column per head so the context matmul
    also yields the softmax denominator; bv is folded into bo on the host
    (bo_eff = bo + bv @ Wo) making the V eviction a pure scalar copy.
  - Attention is emitted at matmul granularity: every scalar-engine exp
    eviction (one [128,2,512] two-bank PSUM tile per key block, both heads)
    is shadowed by an independent dense matmul (next head-pair's QK
    production, context chains, or the denominator broadcast), so the PE
    FIFO never stalls and the HAM clock gate stays at full rate.
  - Softmax denominators: reciprocal_approx_fast on SBUF rows 0/32 (custom
    DVE ops only run at partition base 0), broadcast across partitions with
    two K=1 matmuls in distinct row groups.
  - LN normalizes run on the scalar engine as one Identity activation
    (scale=rsqrt, bias=-mean*rsqrt); LN1 gamma/beta are folded into W1/b1.
  - hT transposes on the PE (lag-2 emission); residual bookkeeping on
    gpsimd; FFN2 evicts into a dedicated accumulator so nothing waits on
    gpsimd; LN2 tail split across scalar/vector/gpsimd.
  - All matmuls bf16 (PSUM accumulate f32); weights converted on host.
"""

import sys

if "/opt/trn_rl_repo" not in sys.path:
    sys.path.insert(0, "/opt/trn_rl_repo")

from contextlib import ExitStack

import ml_dtypes
import numpy as np

import concourse.bass as bass
import concourse.tile as tile
from concourse import bacc, mybir
from concourse.masks import make_identity
from concourse.bass_utils import run_bass_kernel_spmd

F32 = mybir.dt.float32
BF16 = mybir.dt.bfloat16
FP8 = mybir.dt.float8e4
AF = mybir.ActivationFunctionType
ALU = mybir.AluOpType

# Problem dims (hardcoded: nn_BertLayer, hidden 768, 12 heads, ff 3072)
NB = 16
NCORES = 8
BPC = NB // NCORES
S = 512
T = BPC * S
H = 768
HK = H // 128
NH = 12
HD = 64
FF = 3072
EPS = 1e-12
MT = T // 128
NQ = 4           # ffn chunks
FQ = FF // NQ    # 768 ff features per chunk
QK = FQ // 128   # 6 k-tiles per chunk
SCALE = 1.0 / float(np.sqrt(HD))


def _col_ap(vec_ext, ntiles):
    a = vec_ext[:]
    return bass.AP(tensor=a.tensor, offset=a.offset, ap=[[1, 128], [128, ntiles]])


def build_nc():
    nc = bacc.Bacc(num_swdge_queues=4)

    xT_e = nc.declare_dram_parameter("xT", [H, T], BF16, isOutput=False)
    xn_e = nc.declare_dram_parameter("x_bf16", [T, H], BF16, isOutput=False)
    wq_e = nc.declare_dram_parameter("Wq", [H, H], BF16, isOutput=False)
    bq_e = nc.declare_dram_parameter("bq", [H], F32, isOutput=False)
    wk_e = nc.declare_dram_parameter("Wk", [H, H], BF16, isOutput=False)
    bk_e = nc.declare_dram_parameter("bk", [H], F32, isOutput=False)
    wv_e = nc.declare_dram_parameter("Wv", [H, H], BF16, isOutput=False)
    wo_e = nc.declare_dram_parameter("Wo", [H, H], BF16, isOutput=False)
    # pre-broadcast [128, H] vectors (contiguous DMA beats 128 descriptors)
    bo_b = nc.declare_dram_parameter("bo_eff_bc", [128, H], F32, isOutput=False)
    l1g_b = nc.declare_dram_parameter("ln1_g_bc", [128, H], F32, isOutput=False)
    l1b2_b = nc.declare_dram_parameter("lb2_bc", [128, H], F32, isOutput=False)
    w1_e = nc.declare_dram_parameter("W1g", [H, FF], BF16, isOutput=False)
    b1_e = nc.declare_dram_parameter("b1f", [FF], F32, isOutput=False)
    w2_e = nc.declare_dram_parameter("W2", [FF, H], FP8, isOutput=False)
    l2g_b = nc.declare_dram_parameter("ln2_g_bc", [128, H], F32, isOutput=False)
    l2b_b = nc.declare_dram_parameter("ln2_b_bc", [128, H], F32, isOutput=False)
    out_ext = nc.declare_dram_parameter("out", [T, H], F32, isOutput=True)

    with ExitStack() as top:
        tc = top.enter_context(tile.TileContext(nc))

        const = top.enter_context(tc.tile_pool(name="const", bufs=1))
        small = top.enter_context(tc.tile_pool(name="small", bufs=1))
        # ps_sc: 2 bufs x 2 banks (score pairs); ps_ctx: 4 bufs x 1 bank
        ps_sc = top.enter_context(tc.tile_pool(name="ps_sc", bufs=2, space="PSUM"))
        ps_ctx = top.enter_context(tc.tile_pool(name="ps_ctx", bufs=4, space="PSUM"))
        main = top.enter_context(tc.tile_pool(name="main", bufs=1))
        wpool = top.enter_context(tc.tile_pool(name="wpool", bufs=3))

        eps_col = const.tile([128, 1], F32, name="eps_col")
        nc.vector.memset(eps_col, EPS)
        ones_all = const.tile([128, 64], BF16, name="ones_all")
        nc.vector.memset(ones_all, 1.0)
        ident = const.tile([128, 128], BF16, name="ident")
        make_identity(nc, ident)

        # -------- persistent tensors (slots recycled via tags) --------
        xT = main.tile([128, HK, T], BF16, tag="s1", name="xT")
        ctxT = main.tile([128, HK, T], BF16, tag="s2", name="ctxT")
        qT = main.tile([128, HK, T], BF16, tag="s3", bufs=1, name="qT")
        kT = main.tile([128, HK, T], BF16, tag="s4", name="kT")
        vA = main.tile([128, MT, NH, HD + 1], BF16, tag="s5", name="vA")
        nc.vector.memset(vA[:, :, :, HD:HD + 1], 1.0)
        x_nat = main.tile([128, MT, H], BF16, tag="s6n", name="x_nat")

        # ------- input loads: attention-critical tiles first, spread
        # across issuing engines so per-engine queues don't serialize -------
        wvsb = wpool.tile([128, HK, H], BF16, tag="wsb", name="wvsb")
        dma_engs = (nc.sync, nc.scalar, nc.gpsimd)
        for kk in range(HK):
            dma_engs[(2 * kk) % 3].dma_start(
                out=xT[:, kk, :], in_=xT_e[kk * 128:(kk + 1) * 128, :])
            dma_engs[(2 * kk + 1) % 3].dma_start(
                out=wvsb[:, kk, :], in_=wv_e[kk * 128:(kk + 1) * 128, :])

        # ---------------- V natural (dense PE warmup) ----------------
        for mt in range(MT):
            for nt2 in range(2):
                ps = ps_sc.tile([128, 2, 512], F32, tag="ps", name="psv")
                for kk in range(HK):
                    nc.tensor.matmul(
                        ps[:, 0, 0:384],
                        xT[:, kk, mt * 128:(mt + 1) * 128],
                        wvsb[:, kk, nt2 * 384:(nt2 + 1) * 384],
                        start=(kk == 0), stop=(kk == HK - 1),
                    )
                nc.scalar.copy(
                    out=vA[:, mt, nt2 * 6:(nt2 + 1) * 6, 0:HD],
                    in_=ps[:, 0, 0:384].rearrange("p (h d) -> p h d", d=HD),
                )

        wqsb = wpool.tile([128, HK, H], BF16, tag="wsb", name="wqsb")
        for kk in range(HK):
            nc.sync.dma_start(
                out=wqsb[:, kk, :], in_=wq_e[kk * 128:(kk + 1) * 128, :])
        wksb = wpool.tile([128, HK, H], BF16, tag="wsb", name="wksb")
        for kk in range(HK):
            nc.scalar.dma_start(
                out=wksb[:, kk, :], in_=wk_e[kk * 128:(kk + 1) * 128, :])

        # non-urgent loads (needed from the Wo phase on) go after the
        # attention-critical ones so they don't clog the DMA queues
        bq_cols = const.tile([128, HK], F32, name="bq_cols")
        nc.gpsimd.dma_start(out=bq_cols, in_=_col_ap(bq_e, HK))
        bk_cols = const.tile([128, HK], F32, name="bk_cols")
        nc.gpsimd.dma_start(out=bk_cols, in_=_col_ap(bk_e, HK))
        b1_cols = const.tile([128, FF // 128], F32, name="b1_cols")
        nc.gpsimd.dma_start(out=b1_cols, in_=_col_ap(b1_e, FF // 128))

        bo_bc = const.tile([128, H], F32, name="bo_bc")
        nc.gpsimd.dma_start(out=bo_bc, in_=bo_b[:, :])
        l1g_bc = const.tile([128, H], F32, name="l1g_bc")
        nc.gpsimd.dma_start(out=l1g_bc, in_=l1g_b[:, :])
        lb2_bc = const.tile([128, H], F32, name="lb2_bc")
        nc.gpsimd.dma_start(out=lb2_bc, in_=l1b2_b[:, :])
        l2g_bc = const.tile([128, H], F32, name="l2g_bc")
        nc.gpsimd.dma_start(out=l2g_bc, in_=l2g_b[:, :])
        l2b_bc = const.tile([128, H], F32, name="l2b_bc")
        nc.gpsimd.dma_start(out=l2b_bc, in_=l2b_b[:, :])
        for mt in range(MT):
            nc.gpsimd.dma_start(
                out=x_nat[:, mt, :], in_=xn_e[mt * 128:(mt + 1) * 128, :])
            nc.gpsimd.tensor_add(
                out=x_nat[:, mt, :], in0=x_nat[:, mt, :], in1=bo_bc[:])

        # ---------------- attention ----------------
        with ExitStack() as ph_ab:
            expp = ph_ab.enter_context(tc.tile_pool(name="expp", bufs=2))
            bcp = ph_ab.enter_context(tc.tile_pool(name="bcp", bufs=2))

            exp_tiles = {}
            ctx_ps = {}
            live = {}

            def qk_chain(t, i):
                """One of the 4 QK production chains for head-pair t."""
                wsb, b_cols, dstT = ((wqsb, bq_cols, qT),
                                     (wksb, bk_cols, kT))[i // 2]
                nt = i % 2
                ps = ps_sc.tile([128, 2, 512], F32, tag="ps", name="psqk")
                for kk in range(HK):
                    nc.tensor.matmul(
                        ps[:, 0, :],
                        wsb[:, kk, t * 128:(t + 1) * 128],
                        xT[:, kk, nt * 512:(nt + 1) * 512],
                        start=(kk == 0), stop=(kk == HK - 1),
                    )
                nc.vector.tensor_scalar_add(
                    out=dstT[:, t, nt * 512:(nt + 1) * 512],
                    in0=ps[:, 0, :], scalar1=b_cols[:, t:t + 1],
                )

            def qk_pair(t):
                for i in range(4):
                    qk_chain(t, i)

            def s_pair(t, b, kt):
                """Scores for both heads of pair t, key block kt; one exp."""
                if kt == 0:
                    exp_tiles[(t, b)] = expp.tile(
                        [128, 4, 2, 512], BF16, tag="expT", name="expT")
                expT = exp_tiles[(t, b)]
                ps2 = ps_sc.tile([128, 2, 512], F32, tag="ps", name="ps_s2")
                for hh in range(2):
                    poff = hh * 64
                    nc.tensor.matmul(
                        ps2[:, hh, :],
                        kT[poff:poff + 64, t,
                           b * 512 + kt * 128: b * 512 + (kt + 1) * 128],
                        qT[poff:poff + 64, t, b * 512:(b + 1) * 512],
                        start=True, stop=True,
                        tile_position=(poff, 0),
                    )
                nc.scalar.activation(
                    expT[:, kt, :, :], ps2[:], AF.Exp, scale=float(SCALE))

            def ctx_chain(t, b, hh):
                expT = exp_tiles[(t, b)]
                h = 2 * t + hh
                ps_c = ps_ctx.tile([HD + 1, 512], F32, tag="ctx", name="ps_c")
                for kt in range(4):
                    nc.tensor.matmul(
                        ps_c,
                        vA[:, b * 4 + kt, h, :],
                        expT[:, kt, hh, :],
                        start=(kt == 0), stop=(kt == 3),
                    )
                ctx_ps.setdefault((t, b), [None, None])[hh] = ps_c

            def den_chain(t, b):
                """Denominator rows -> SBUF rows 0/32 -> approx recip."""
                pcs = ctx_ps.pop((t, b))
                den2 = bcp.tile([128, 512], F32, tag="den", name="den2")
                nc.scalar.copy(out=den2[0:1, :], in_=pcs[0][HD:HD + 1, :])
                nc.scalar.copy(out=den2[32:33, :], in_=pcs[1][HD:HD + 1, :])
                rbf = bcp.tile([128, 512], F32, tag="rbf", name="rbf")
                nc.vector.reciprocal_approx_fast(
                    out=rbf[0:64, :], in_=den2[0:64, :])
                rb2 = bcp.tile([128, 512], BF16, tag="rb2", name="rb2")
                nc.scalar.copy(out=rb2[0:33, :], in_=rbf[0:33, :])
                live[(t, b)] = (pcs, rb2)

            def nm_bcast(t, b):
                """Broadcast the reciprocals across partitions: both heads
                into ONE psum bank (head 1 lands on partitions 64..127 via
                col-group tiling), evicted with a single DVE copy."""
                pcs, rb2 = live[(t, b)]
                bc_sb = bcp.tile([128, 512], BF16, tag="bc", name="bc_sb")
                ps_b = ps_sc.tile([128, 512], F32, tag="ps", name="ps_b")
                nc.tensor.matmul(
                    ps_b[0:64, :], ones_all[0:1, :],
                    rb2[0:1, :], start=True, stop=True)
                nc.tensor.matmul(
                    ps_b[64:128, :], ones_all[32:33, :],
                    rb2[32:33, :], start=True, stop=True)
                nc.vector.tensor_copy(out=bc_sb[:], in_=ps_b[:])
                live[(t, b)] = (pcs, bc_sb)

            def nm_mul(t, b):
                pcs, bc_sb = live.pop((t, b))
                for hh in range(2):
                    poff = hh * 64
                    nc.vector.tensor_mul(
                        out=ctxT[poff:poff + 64, t, b * 512:(b + 1) * 512],
                        in0=pcs[hh][0:64, :], in1=bc_sb[poff:poff + 64, :],
                    )

            # --- software pipeline: every exp is shadowed by dense MMs ---
            qk_pair(0)
            for t in range(HK):
                s_pair(t, 0, 0)
                if t < HK - 1:
                    qk_chain(t + 1, 0)
                s_pair(t, 0, 1)
                if t > 0:
                    nm_bcast(t - 1, 0)
                s_pair(t, 0, 2)
                if t < HK - 1:
                    qk_chain(t + 1, 1)
                if t > 0:
                    nm_mul(t - 1, 0)
                s_pair(t, 0, 3)
                if t > 0:
                    nm_bcast(t - 1, 1)
                s_pair(t, 1, 0)
                if t < HK - 1:
                    qk_chain(t + 1, 2)
                if t > 0:
                    nm_mul(t - 1, 1)
                s_pair(t, 1, 1)
                if t < HK - 1:
                    qk_chain(t + 1, 3)
                ctx_chain(t, 0, 0)
                s_pair(t, 1, 2)
                ctx_chain(t, 0, 1)
                den_chain(t, 0)
                s_pair(t, 1, 3)
                ctx_chain(t, 1, 0)
                ctx_chain(t, 1, 1)
                den_chain(t, 1)
            nm_bcast(HK - 1, 0)
            nm_mul(HK - 1, 0)

            # ------------- Wo + residual + LN1 + h transpose -------------
            # hT reuses xT's slot; acc reuses kT's.
            hT = main.tile([128, HK, T], BF16, tag="s1", name="hT")
            acc = main.tile([128, MT, H], F32, tag="s4", name="acc")
            with tc.tile_pool(name="attp", bufs=4) as attp:
                wosb = wpool.tile([128, HK, H], BF16, tag="wsb", name="wosb")
                for kk in range(HK):
                    nc.sync.dma_start(
                        out=wosb[:, kk, :], in_=wo_e[kk * 128:(kk + 1) * 128, :])

                hbs = {}

                def emit_transposes(mt):
                    hb = hbs.pop(mt)
                    for c in range(0, HK, 2):
                        pt = ps_sc.tile([128, 2, 128], BF16, tag="ps",
                                        name="pt")
                        for j in range(2):
                            nc.tensor.transpose(
                                pt[:, j, :],
                                hb[:, (c + j) * 128:(c + j + 1) * 128], ident)
                        nc.scalar.copy(
                            out=hT[:, c:c + 2, mt * 128:(mt + 1) * 128],
                            in_=pt[:])

                for mt in range(MT):
                    if mt == 1:
                        # rest of the attention epilogue, overlapped with
                        # the first Wo matmuls (they only need batch 0)
                        nm_bcast(HK - 1, 1)
                        nm_mul(HK - 1, 1)
                    # transposes lag two iterations so the PE never waits on
                    # the LN chain
                    if mt >= 2:
                        emit_transposes(mt - 2)
                    attn = attp.tile([128, H], F32, tag="attn", name="attn")
                    for nt2 in range(2):
                        ps = ps_ctx.tile([128, 384], F32, tag="ctx", name="psw")
                        for kk in range(HK):
                            nc.tensor.matmul(
                                ps,
                                ctxT[:, kk, mt * 128:(mt + 1) * 128],
                                wosb[:, kk, nt2 * 384:(nt2 + 1) * 384],
                                start=(kk == 0), stop=(kk == HK - 1),
                            )
                        nc.vector.tensor_add(
                            out=attn[:, nt2 * 384:(nt2 + 1) * 384],
                            in0=ps[:], in1=x_nat[:, mt, nt2 * 384:(nt2 + 1) * 384])
                    # LN1 -> z in bf16 (gamma/beta folded into W1/b1); the
                    # normalize itself is ONE scalar-engine Identity activation
                    st = small.tile([128, 2, 6], F32, tag="lnst", bufs=8, name="st")
                    for i in range(2):
                        nc.vector.bn_stats(out=st[:, i, :],
                                           in_=attn[:, i * 384:(i + 1) * 384])
                    mv = small.tile([128, 2], F32, tag="lnmv", bufs=8, name="mv")
                    nc.vector.bn_aggr(out=mv[:], in_=st[:])
                    sd = small.tile([128, 1], F32, tag="lnsd", bufs=8, name="sd")
                    nc.scalar.activation(sd[:], mv[:, 1:2], AF.Abs_reciprocal_sqrt,
                                         bias=eps_col[:])
                    msd = small.tile([128, 1], F32, tag="lnms", bufs=8, name="msd")
                    nc.vector.tensor_scalar(
                        out=msd[:], in0=mv[:, 0:1], scalar1=sd[:], scalar2=-1.0,
                        op0=ALU.mult, op1=ALU.mult)
                    hb = attp.tile([128, H], BF16, tag="hb", name="hb")
                    nc.scalar.activation(hb[:], attn[:], AF.Identity,
                                         scale=sd[:], bias=msd[:])
                    hbs[mt] = hb
                    # residual path: acc = z*g1 + (ln1_b + b2)  (gpsimd, off the
                    # critical path — FFN2 no longer waits on it)
                    nc.gpsimd.tensor_mul(acc[:, mt, :], hb[:], l1g_bc[:])
                    nc.gpsimd.tensor_add(acc[:, mt, :], acc[:, mt, :], lb2_bc[:])
                emit_transposes(MT - 2)
                emit_transposes(MT - 1)

        # ---------------- FFN ----------------
        # gT_all holds the WHOLE gelu output in fp8 (reuses vA's slot) and
        # W2 stays resident, so FFN2 runs mt-major: the full 3072-dim
        # contraction accumulates in one PSUM bank per output tile (a single
        # eviction), and LN2 follows each mt — the post-matmul drain is just
        # the final token block's chain.
        gT_all = main.tile([128, FF // 128, T], FP8, tag="s5", name="gT_all")
        w2a = wpool.tile([128, FF // 128, H], FP8, tag="w2a", bufs=1,
                         name="w2a")
        for kk in range(FF // 128):
            eng = (nc.sync, nc.scalar)[kk % 2]
            eng.dma_start(out=w2a[:, kk, :],
                          in_=w2_e[kk * 128:(kk + 1) * 128, :])
        with tc.tile_pool(name="outp", bufs=3) as outp:
            for q in range(NQ):
                w1c = wpool.tile([128, HK, FQ], BF16, tag="wsb", name="w1c")
                for kk in range(HK):
                    nc.sync.dma_start(
                        out=w1c[:, kk, :],
                        in_=w1_e[kk * 128:(kk + 1) * 128, q * FQ:(q + 1) * FQ])
                for nt in range(2):
                    for mo in range(QK):
                        ps = ps_ctx.tile([128, 512], F32, tag="ctx",
                                         name="psf1")
                        for kk in range(HK):
                            nc.tensor.matmul(
                                ps,
                                w1c[:, kk, mo * 128:(mo + 1) * 128],
                                hT[:, kk, nt * 512:(nt + 1) * 512],
                                start=(kk == 0), stop=(kk == HK - 1),
                            )
                        f = q * QK + mo
                        nc.scalar.activation(
                            gT_all[:, f, nt * 512:(nt + 1) * 512], ps[:],
                            AF.Gelu, bias=b1_cols[:, f:f + 1])
            for mt in range(MT):
                for nt2 in range(2):
                    ps = ps_ctx.tile([128, 384], F32, tag="ctx", name="psf2")
                    for kk in range(FF // 256):
                        nc.tensor.matmul(
                            ps,
                            gT_all[:, 2 * kk:2 * kk + 2,
                                   mt * 128:(mt + 1) * 128],
                            w2a[:, 2 * kk:2 * kk + 2,
                                nt2 * 384:(nt2 + 1) * 384],
                            start=(kk == 0), stop=(kk == FF // 256 - 1),
                            perf_mode=mybir.MatmulPerfMode.DoubleRow,
                        )
                    nc.vector.tensor_add(
                        out=acc[:, mt, nt2 * 384:(nt2 + 1) * 384],
                        in0=acc[:, mt, nt2 * 384:(nt2 + 1) * 384],
                        in1=ps[:])
                # ---- LN2 + store ----
                src_ = acc[:, mt, :]
                st = small.tile([128, 2, 6], F32, tag="lnst", bufs=8,
                                name="st2")
                for i in range(2):
                    nc.vector.bn_stats(out=st[:, i, :],
                                       in_=src_[:, i * 384:(i + 1) * 384])
                mv = small.tile([128, 2], F32, tag="lnmv", bufs=8, name="mv2")
                nc.vector.bn_aggr(out=mv[:], in_=st[:])
                sd = small.tile([128, 1], F32, tag="lnsd", bufs=8, name="sd2")
                nc.scalar.activation(sd[:], mv[:, 1:2],
                                     AF.Abs_reciprocal_sqrt, bias=eps_col[:])
                msd = small.tile([128, 1], F32, tag="lnms", bufs=8,
                                 name="msd2")
                nc.vector.tensor_scalar(
                    out=msd[:], in0=mv[:, 0:1], scalar1=sd[:],
                    scalar2=-1.0, op0=ALU.mult, op1=ALU.mult)
                ot = outp.tile([128, H], F32, tag="ot", name="ot")
                nc.scalar.activation(ot[:], src_, AF.Identity,
                                     scale=sd[:], bias=msd[:])
                # gamma on DVE, beta alternates gpsimd/DVE
                nc.vector.tensor_mul(ot[:], ot[:], l2g_bc[:])
                eng_b = nc.gpsimd if mt % 2 == 1 else nc.vector
                eng_b.tensor_add(ot[:], ot[:], l2b_bc[:])
                nc.sync.dma_start(
                    out=out_ext[mt * 128:(mt + 1) * 128, :], in_=ot)

    nc.finalize()
    return nc


_NC = None


def _get_nc():
    global _NC
    if _NC is None:
        _NC = build_nc()
    return _NC


def run(inputs, trace=False):
    f32 = lambda n: np.ascontiguousarray(np.asarray(inputs[n], dtype=np.float32))

    def bf16(a):
        return np.ascontiguousarray(a.astype(ml_dtypes.bfloat16))

    hs = f32("hidden_state").reshape(NB, S, H)
    w1 = f32("W1")
    wo = f32("Wo")
    l1g = f32("ln1_g")
    l1b = f32("ln1_b")

    def bc128(v):
        return np.ascontiguousarray(np.broadcast_to(v, (128, H)))

    common = {
        "Wq": bf16(f32("Wq")), "bq": f32("bq"),
        "Wk": bf16(f32("Wk")), "bk": f32("bk"),
        "Wv": bf16(f32("Wv")),
        "Wo": bf16(wo),
        # fold the V bias through Wo:  softmax rows sum to 1
        "bo_eff_bc": bc128(f32("bo") + f32("bv") @ wo),
        "ln1_g_bc": bc128(l1g),
        "lb2_bc": bc128(l1b + f32("b2")),
        # fold LN1 gamma/beta into the FFN input projection
        "W1g": bf16(l1g[:, None] * w1),
        "b1f": np.ascontiguousarray(f32("b1") + l1b @ w1),
        "W2": np.ascontiguousarray(
            np.clip(f32("W2"), -240, 240).astype(ml_dtypes.float8_e4m3fn)),
        "ln2_g_bc": bc128(f32("ln2_g")), "ln2_b_bc": bc128(f32("ln2_b")),
    }
    in_maps = []
    for i in range(NCORES):
        m = dict(common)
        x = np.ascontiguousarray(hs[i * BPC:(i + 1) * BPC].reshape(T, H))
        m["x_bf16"] = bf16(x)
        m["xT"] = bf16(x.T)
        in_maps.append(m)
    res = run_bass_kernel_spmd(_get_nc(), in_maps, core_ids=list(range(NCORES)),
                               trace=trace)
    out = np.concatenate(
        [res.results[i]["out"].reshape(BPC, S, H) for i in range(NCORES)], axis=0)
    return out, res


def kernel(**inputs):
    return run(inputs)[0]



# revision 2
# speedup vs baseline: 1.1596x; 1.1596x over previous
"""BertLayer on 8 trn2 NeuronCores — data-parallel over batch (2 per core).

v9: full-fp8 matmul path (e4m3 + DoubleRow, K=256 per instruction) on top of
the v8 layout:
  - Every GEMM operand is fp8: xT, Wq/Wk/Wv (x64), qT/kT (64x), vA (8x),
    expT, ctxT (8x), Wo (32x), hT, W1g (64x), gT, W2 (2048x).  Scale factors
    are powers of two folded host-side; LN1/LN2 are scale-invariant so the
    residual paths run scaled (x_nat at 256x bf16, acc at 2048x f32) and no
    descale op is ever needed.  exp descale rides the activation scale.
  - DoubleRow on V/QKV/ctx/Wo/FFN1/FFN2 halves PE instruction count; scores
    stay normal-mode fp8 (K=64/head).  vA pads each head to 68 cols so the
    kt-pair stationary AP step (12*68=816B) is 16B-aligned for DoubleRow.
  - Softmax denominator still rides the ctx matmul via a ones column in vA;
    reciprocal on DVE rows 0/32, broadcast with two K=1 matmuls.
  - den-row PSUM->SBUF staging copies moved to DVE (scalar engine is the
    attention-phase bottleneck: 8 exp evictions per t-iteration).
  - LN normalizes on scalar as one Identity activation; LN1 gamma/beta folded
    into W1/b1; W2 scaled 2048x so its fp8 encoding avoids the subnormal
    range (halves the FFN2 quantization error).
"""

import sys

if "/opt/trn_rl_repo" not in sys.path:
    sys.path.insert(0, "/opt/trn_rl_repo")

from contextlib import ExitStack

import ml_dtypes
import numpy as np

import concourse.bass as bass
import concourse.tile as tile
from concourse import bacc, mybir
from concourse.masks import make_identity
from concourse.bass_utils import run_bass_kernel_spmd

F32 = mybir.dt.float32
BF16 = mybir.dt.bfloat16
FP8 = mybir.dt.float8e4
AF = mybir.ActivationFunctionType
ALU = mybir.AluOpType
DR = mybir.MatmulPerfMode.DoubleRow

# Problem dims (hardcoded: nn_BertLayer, hidden 768, 12 heads, ff 3072)
NB = 16
NCORES = 8
BPC = NB // NCORES
S = 512
T = BPC * S
H = 768
HK = H // 128
NH = 12
HD = 64
HP = HD + 4      # padded per-head stride in vA: 12*68 = 816 B, 16B-aligned
FF = 3072
EPS = 1e-12
MT = T // 128
NQ = 4           # ffn chunks
FQ = FF // NQ    # 768 ff features per chunk
QK = FQ // 128   # 6 k-tiles per chunk
SCALE = 1.0 / float(np.sqrt(HD))

# fp8 scale plan (powers of two; LN scale-invariance absorbs them)
SQK = 64.0            # Wq*,Wk* = 64 W; qT,kT hold 64q, 64k
SV = 8.0              # vA holds 8 v   (Wv* = 64 Wv, evict scale 1/8)
SWO = 32.0            # Wo* = 32 Wo
SA = SV * SWO         # attn psum = 256 attn_out; x_nat = 256 x
SW1 = 64.0            # W1g* = 64 l1g W1; gelu activation scale 1/64
SW2 = 2048.0          # W2* = 2048 W2; acc carries 2048x
EXPSCALE = SCALE / (SQK * SQK)


def _col_ap(vec_ext, ntiles):
    a = vec_ext[:]
    return bass.AP(tensor=a.tensor, offset=a.offset, ap=[[1, 128], [128, ntiles]])


def build_nc():
    nc = bacc.Bacc(num_swdge_queues=4)

    xT_e = nc.declare_dram_parameter("xT", [H, T], FP8, isOutput=False)
    xn_e = nc.declare_dram_parameter("x_bf16", [T, H], BF16, isOutput=False)
    wq_e = nc.declare_dram_parameter("Wq", [H, H], FP8, isOutput=False)
    bq_e = nc.declare_dram_parameter("bq", [H], F32, isOutput=False)
    wk_e = nc.declare_dram_parameter("Wk", [H, H], FP8, isOutput=False)
    bk_e = nc.declare_dram_parameter("bk", [H], F32, isOutput=False)
    wv_e = nc.declare_dram_parameter("Wv", [H, H], FP8, isOutput=False)
    wo_e = nc.declare_dram_parameter("Wo", [H, H], FP8, isOutput=False)
    # pre-broadcast [128, H] vectors (contiguous DMA beats 128 descriptors)
    bo_b = nc.declare_dram_parameter("bo_eff_bc", [128, H], F32, isOutput=False)
    l1g_b = nc.declare_dram_parameter("ln1_g_bc", [128, H], F32, isOutput=False)
    l1b2_b = nc.declare_dram_parameter("lb2_bc", [128, H], F32, isOutput=False)
    w1_e = nc.declare_dram_parameter("W1g", [H, FF], FP8, isOutput=False)
    b1_e = nc.declare_dram_parameter("b1f", [FF], F32, isOutput=False)
    w2_e = nc.declare_dram_parameter("W2", [FF, H], FP8, isOutput=False)
    l2g_b = nc.declare_dram_parameter("ln2_g_bc", [128, H], F32, isOutput=False)
    l2b_b = nc.declare_dram_parameter("ln2_b_bc", [128, H], F32, isOutput=False)
    out_ext = nc.declare_dram_parameter("out", [T, H], F32, isOutput=True)

    with ExitStack() as top:
        tc = top.enter_context(tile.TileContext(nc))

        const = top.enter_context(tc.tile_pool(name="const", bufs=1))
        small = top.enter_context(tc.tile_pool(name="small", bufs=1))
        # ps_sc: 2 bufs x 2 banks (score pairs); ps_ctx: 4 bufs x 1 bank
        ps_sc = top.enter_context(tc.tile_pool(name="ps_sc", bufs=2, space="PSUM"))
        ps_ctx = top.enter_context(tc.tile_pool(name="ps_ctx", bufs=4, space="PSUM"))
        main = top.enter_context(tc.tile_pool(name="main", bufs=1))
        wpool = top.enter_context(tc.tile_pool(name="wpool", bufs=3))

        eps_col = const.tile([128, 1], F32, name="eps_col")
        nc.vector.memset(eps_col, EPS)
        ones_all = const.tile([128, 64], BF16, name="ones_all")
        nc.vector.memset(ones_all, 1.0)
        ident = const.tile([128, 128], BF16, name="ident")
        make_identity(nc, ident)

        # -------- persistent tensors (slots recycled via tags) --------
        xT = main.tile([128, HK, T], FP8, tag="s1", name="xT")
        ctxT = main.tile([128, HK, T], FP8, tag="s2", name="ctxT")
        qT = main.tile([128, HK, T], FP8, tag="s3", bufs=1, name="qT")
        kT = main.tile([128, HK, T], FP8, tag="s4", name="kT")
        vA = main.tile([128, MT, NH, HP], FP8, tag="s5", name="vA")
        nc.vector.memset(vA[:, :, :, HD:HD + 1], 1.0)
        x_nat = main.tile([128, MT, H], BF16, tag="s6n", name="x_nat")

        # ------- input loads: attention-critical tiles first, spread
        # across issuing engines so per-engine queues don't serialize -------
        wvsb = wpool.tile([128, HK, H], FP8, tag="wsb", name="wvsb")
        dma_engs = (nc.sync, nc.scalar, nc.gpsimd)
        for kk in range(HK):
            dma_engs[(2 * kk) % 3].dma_start(
                out=xT[:, kk, :], in_=xT_e[kk * 128:(kk + 1) * 128, :])
            dma_engs[(2 * kk + 1) % 3].dma_start(
                out=wvsb[:, kk, :], in_=wv_e[kk * 128:(kk + 1) * 128, :])

        # ---------------- V natural (dense PE warmup) ----------------
        for mt in range(MT):
            for nt2 in range(2):
                ps = ps_sc.tile([128, 2, 512], F32, tag="ps", name="psv")
                for kp in range(HK // 2):
                    nc.tensor.matmul(
                        ps[:, 0, 0:384],
                        xT[:, 2 * kp:2 * kp + 2, mt * 128:(mt + 1) * 128],
                        wvsb[:, 2 * kp:2 * kp + 2, nt2 * 384:(nt2 + 1) * 384],
                        start=(kp == 0), stop=(kp == HK // 2 - 1),
                        perf_mode=DR,
                    )
                nc.scalar.activation(
                    vA[:, mt, nt2 * 6:(nt2 + 1) * 6, 0:HD],
                    ps[:, 0, 0:384].rearrange("p (h d) -> p h d", d=HD),
                    AF.Copy, scale=float(1.0 / SV),
                )

        wqsb = wpool.tile([128, HK, H], FP8, tag="wsb", name="wqsb")
        for kk in range(HK):
            nc.sync.dma_start(
                out=wqsb[:, kk, :], in_=wq_e[kk * 128:(kk + 1) * 128, :])
        wksb = wpool.tile([128, HK, H], FP8, tag="wsb", name="wksb")
        for kk in range(HK):
            nc.scalar.dma_start(
                out=wksb[:, kk, :], in_=wk_e[kk * 128:(kk + 1) * 128, :])

        # non-urgent loads (needed from the Wo phase on) go after the
        # attention-critical ones so they don't clog the DMA queues
        bq_cols = const.tile([128, HK], F32, name="bq_cols")
        nc.gpsimd.dma_start(out=bq_cols, in_=_col_ap(bq_e, HK))
        bk_cols = const.tile([128, HK], F32, name="bk_cols")
        nc.gpsimd.dma_start(out=bk_cols, in_=_col_ap(bk_e, HK))
        b1_cols = const.tile([128, FF // 128], F32, name="b1_cols")
        nc.gpsimd.dma_start(out=b1_cols, in_=_col_ap(b1_e, FF // 128))

        bo_bc = const.tile([128, H], F32, name="bo_bc")
        nc.gpsimd.dma_start(out=bo_bc, in_=bo_b[:, :])
        l1g_bc = const.tile([128, H], F32, name="l1g_bc")
        nc.gpsimd.dma_start(out=l1g_bc, in_=l1g_b[:, :])
        lb2_bc = const.tile([128, H], F32, name="lb2_bc")
        nc.gpsimd.dma_start(out=lb2_bc, in_=l1b2_b[:, :])
        l2g_bc = const.tile([128, H], F32, name="l2g_bc")
        nc.gpsimd.dma_start(out=l2g_bc, in_=l2g_b[:, :])
        l2b_bc = const.tile([128, H], F32, name="l2b_bc")
        nc.gpsimd.dma_start(out=l2b_bc, in_=l2b_b[:, :])
        for mt in range(MT):
            nc.gpsimd.dma_start(
                out=x_nat[:, mt, :], in_=xn_e[mt * 128:(mt + 1) * 128, :])
            nc.gpsimd.tensor_add(
                out=x_nat[:, mt, :], in0=x_nat[:, mt, :], in1=bo_bc[:])

        # ---------------- attention ----------------
        with ExitStack() as ph_ab:
            expp = ph_ab.enter_context(tc.tile_pool(name="expp", bufs=2))
            bcp = ph_ab.enter_context(tc.tile_pool(name="bcp", bufs=2))

            exp_tiles = {}
            ctx_ps = {}
            live = {}

            def qk_chain(t, i):
                """One of the 4 QK production chains for head-pair t."""
                wsb, b_cols, dstT = ((wqsb, bq_cols, qT),
                                     (wksb, bk_cols, kT))[i // 2]
                nt = i % 2
                ps = ps_sc.tile([128, 2, 512], F32, tag="ps", name="psqk")
                for kp in range(HK // 2):
                    nc.tensor.matmul(
                        ps[:, 0, :],
                        wsb[:, 2 * kp:2 * kp + 2, t * 128:(t + 1) * 128],
                        xT[:, 2 * kp:2 * kp + 2, nt * 512:(nt + 1) * 512],
                        start=(kp == 0), stop=(kp == HK // 2 - 1),
                        perf_mode=DR,
                    )
                nc.vector.tensor_scalar_add(
                    out=dstT[:, t, nt * 512:(nt + 1) * 512],
                    in0=ps[:, 0, :], scalar1=b_cols[:, t:t + 1],
                )

            def qk_pair(t):
                for i in range(4):
                    qk_chain(t, i)

            def s_pair(t, b, kt):
                """Scores for both heads of pair t, key block kt; one exp."""
                if kt == 0:
                    exp_tiles[(t, b)] = expp.tile(
                        [128, 4, 2, 512], FP8, tag="expT", name="expT")
                expT = exp_tiles[(t, b)]
                ps2 = ps_sc.tile([128, 2, 512], F32, tag="ps", name="ps_s2")
                for hh in range(2):
                    poff = hh * 64
                    nc.tensor.matmul(
                        ps2[:, hh, :],
                        kT[poff:poff + 64, t,
                           b * 512 + kt * 128: b * 512 + (kt + 1) * 128],
                        qT[poff:poff + 64, t, b * 512:(b + 1) * 512],
                        start=True, stop=True,
                        tile_position=(poff, 0),
                    )
                nc.scalar.activation(
                    expT[:, kt, :, :], ps2[:], AF.Exp, scale=float(EXPSCALE))

            def ctx_chain(t, b, hh):
                expT = exp_tiles[(t, b)]
                h = 2 * t + hh
                ps_c = ps_ctx.tile([HD + 1, 512], F32, tag="ctx", name="ps_c")
                for kp in range(2):
                    nc.tensor.matmul(
                        ps_c,
                        vA[:, b * 4 + 2 * kp:b * 4 + 2 * kp + 2, h, 0:HD + 1],
                        expT[:, 2 * kp:2 * kp + 2, hh, :],
                        start=(kp == 0), stop=(kp == 1),
                        perf_mode=DR,
                    )
                ctx_ps.setdefault((t, b), [None, None])[hh] = ps_c

            def den_chain(t, b):
                """Denominator rows -> SBUF rows 0/32 -> approx recip."""
                pcs = ctx_ps.pop((t, b))
                den2 = bcp.tile([128, 512], F32, tag="den", name="den2")
                nc.vector.tensor_copy(out=den2[0:1, :], in_=pcs[0][HD:HD + 1, :])
                nc.vector.tensor_copy(out=den2[32:33, :], in_=pcs[1][HD:HD + 1, :])
                rbf = bcp.tile([128, 512], F32, tag="rbf", name="rbf")
                nc.vector.reciprocal_approx_fast(
                    out=rbf[0:64, :], in_=den2[0:64, :])
                rb2 = bcp.tile([128, 512], BF16, tag="rb2", name="rb2")
                nc.scalar.copy(out=rb2[0:33, :], in_=rbf[0:33, :])
                live[(t, b)] = (pcs, rb2)

            def nm_bcast(t, b):
                """Broadcast the reciprocals across partitions: both heads
                into ONE psum bank (head 1 lands on partitions 64..127 via
                col-group tiling), evicted with a single DVE copy."""
                pcs, rb2 = live[(t, b)]
                bc_sb = bcp.tile([128, 512], BF16, tag="bc", name="bc_sb")
                ps_b = ps_sc.tile([128, 512], F32, tag="ps", name="ps_b")
                nc.tensor.matmul(
                    ps_b[0:64, :], ones_all[0:1, :],
                    rb2[0:1, :], start=True, stop=True)
                nc.tensor.matmul(
                    ps_b[64:128, :], ones_all[32:33, :],
                    rb2[32:33, :], start=True, stop=True)
                nc.vector.tensor_copy(out=bc_sb[:], in_=ps_b[:])
                live[(t, b)] = (pcs, bc_sb)

            def nm_mul(t, b):
                pcs, bc_sb = live.pop((t, b))
                for hh in range(2):
                    poff = hh * 64
                    nc.vector.tensor_mul(
                        out=ctxT[poff:poff + 64, t, b * 512:(b + 1) * 512],
                        in0=pcs[hh][0:64, :], in1=bc_sb[poff:poff + 64, :],
                    )

            # --- software pipeline: every exp is shadowed by dense MMs ---
            qk_pair(0)
            for t in range(HK):
                s_pair(t, 0, 0)
                if t < HK - 1:
                    qk_chain(t + 1, 0)
                s_pair(t, 0, 1)
                if t > 0:
                    nm_bcast(t - 1, 0)
                s_pair(t, 0, 2)
                if t < HK - 1:
                    qk_chain(t + 1, 1)
                if t > 0:
                    nm_mul(t - 1, 0)
                s_pair(t, 0, 3)
                if t > 0:
                    nm_bcast(t - 1, 1)
                s_pair(t, 1, 0)
                if t < HK - 1:
                    qk_chain(t + 1, 2)
                if t > 0:
                    nm_mul(t - 1, 1)
                s_pair(t, 1, 1)
                if t < HK - 1:
                    qk_chain(t + 1, 3)
                ctx_chain(t, 0, 0)
                s_pair(t, 1, 2)
                ctx_chain(t, 0, 1)
                den_chain(t, 0)
                s_pair(t, 1, 3)
                ctx_chain(t, 1, 0)
                ctx_chain(t, 1, 1)
                den_chain(t, 1)
            nm_bcast(HK - 1, 0)
            nm_mul(HK - 1, 0)

            # ------------- Wo + residual + LN1 + h transpose -------------
            # hT reuses xT's slot; acc reuses kT's.
            hT = main.tile([128, HK, T], FP8, tag="s1", name="hT")
            acc = main.tile([128, MT, H], F32, tag="s4", name="acc")
            with tc.tile_pool(name="attp", bufs=4) as attp:
                wosb = wpool.tile([128, HK, H], FP8, tag="wsb", name="wosb")
                for kk in range(HK):
                    nc.sync.dma_start(
                        out=wosb[:, kk, :], in_=wo_e[kk * 128:(kk + 1) * 128, :])

                hbs = {}

                def emit_transposes(mt):
                    hb = hbs.pop(mt)
                    for c in range(0, HK, 2):
                        pt = ps_sc.tile([128, 2, 128], BF16, tag="ps",
                                        name="pt")
                        for j in range(2):
                            nc.tensor.transpose(
                                pt[:, j, :],
                                hb[:, (c + j) * 128:(c + j + 1) * 128], ident)
                        nc.scalar.copy(
                            out=hT[:, c:c + 2, mt * 128:(mt + 1) * 128],
                            in_=pt[:])

                for mt in range(MT):
                    if mt == 1:
                        # rest of the attention epilogue, overlapped with
                        # the first Wo matmuls (they only need batch 0)
                        nm_bcast(HK - 1, 1)
                        nm_mul(HK - 1, 1)
                    # transposes lag two iterations so the PE never waits on
                    # the LN chain
                    if mt >= 2:
                        emit_transposes(mt - 2)
                    attn = attp.tile([128, H], F32, tag="attn", name="attn")
                    for nt2 in range(2):
                        ps = ps_ctx.tile([128, 384], F32, tag="ctx", name="psw")
                        for kp in range(HK // 2):
                            nc.tensor.matmul(
                                ps,
                                ctxT[:, 2 * kp:2 * kp + 2, mt * 128:(mt + 1) * 128],
                                wosb[:, 2 * kp:2 * kp + 2, nt2 * 384:(nt2 + 1) * 384],
                                start=(kp == 0), stop=(kp == HK // 2 - 1),
                                perf_mode=DR,
                            )
                        nc.vector.tensor_add(
                            out=attn[:, nt2 * 384:(nt2 + 1) * 384],
                            in0=ps[:], in1=x_nat[:, mt, nt2 * 384:(nt2 + 1) * 384])
                    # LN1 -> z in bf16 (gamma/beta folded into W1/b1); the
                    # normalize itself is ONE scalar-engine Identity activation
                    st = small.tile([128, 2, 6], F32, tag="lnst", bufs=8, name="st")
                    for i in range(2):
                        nc.vector.bn_stats(out=st[:, i, :],
                                           in_=attn[:, i * 384:(i + 1) * 384])
                    mv = small.tile([128, 2], F32, tag="lnmv", bufs=8, name="mv")
                    nc.vector.bn_aggr(out=mv[:], in_=st[:])
                    sd = small.tile([128, 1], F32, tag="lnsd", bufs=8, name="sd")
                    nc.scalar.activation(sd[:], mv[:, 1:2], AF.Abs_reciprocal_sqrt,
                                         bias=eps_col[:])
                    msd = small.tile([128, 1], F32, tag="lnms", bufs=8, name="msd")
                    nc.vector.tensor_scalar(
                        out=msd[:], in0=mv[:, 0:1], scalar1=sd[:], scalar2=-1.0,
                        op0=ALU.mult, op1=ALU.mult)
                    hb = attp.tile([128, H], BF16, tag="hb", name="hb")
                    nc.scalar.activation(hb[:], attn[:], AF.Identity,
                                         scale=sd[:], bias=msd[:])
                    hbs[mt] = hb
                    # residual path: acc = z*g1 + (ln1_b + b2), all 2048x
                    # (gpsimd, off the critical path)
                    nc.gpsimd.tensor_mul(acc[:, mt, :], hb[:], l1g_bc[:])
                    nc.gpsimd.tensor_add(acc[:, mt, :], acc[:, mt, :], lb2_bc[:])
                emit_transposes(MT - 2)
                emit_transposes(MT - 1)

        # ---------------- FFN ----------------
        # gT_all holds the WHOLE gelu output in fp8 (reuses vA's slot) and
        # W2 stays resident, so FFN2 runs mt-major: the full 3072-dim
        # contraction accumulates in one PSUM bank per output tile (a single
        # eviction), and LN2 follows each mt — the post-matmul drain is just
        # the final token block's chain.
        gT_all = main.tile([128, FF // 128, T], FP8, tag="s5", name="gT_all")
        w2a = wpool.tile([128, FF // 128, H], FP8, tag="w2a", bufs=1,
                         name="w2a")
        for kk in range(FF // 128):
            eng = (nc.sync, nc.scalar)[kk % 2]
            eng.dma_start(out=w2a[:, kk, :],
                          in_=w2_e[kk * 128:(kk + 1) * 128, :])
        with tc.tile_pool(name="outp", bufs=3) as outp:
            for q in range(NQ):
                w1c = wpool.tile([128, HK, FQ], FP8, tag="wsb", name="w1c")
                for kk in range(HK):
                    nc.sync.dma_start(
                        out=w1c[:, kk, :],
                        in_=w1_e[kk * 128:(kk + 1) * 128, q * FQ:(q + 1) * FQ])
                for nt in range(2):
                    for mo in range(QK):
                        ps = ps_ctx.tile([128, 512], F32, tag="ctx",
                                         name="psf1")
                        for kp in range(HK // 2):
                            nc.tensor.matmul(
                                ps,
                                w1c[:, 2 * kp:2 * kp + 2, mo * 128:(mo + 1) * 128],
                                hT[:, 2 * kp:2 * kp + 2, nt * 512:(nt + 1) * 512],
                                start=(kp == 0), stop=(kp == HK // 2 - 1),
                                perf_mode=DR,
                            )
                        f = q * QK + mo
                        nc.scalar.activation(
                            gT_all[:, f, nt * 512:(nt + 1) * 512], ps[:],
                            AF.Gelu, bias=b1_cols[:, f:f + 1],
                            scale=float(1.0 / SW1))
            for mt in range(MT):
                for nt2 in range(2):
                    ps = ps_ctx.tile([128, 384], F32, tag="ctx", name="psf2")
                    for kk in range(FF // 256):
                        nc.tensor.matmul(
                            ps,
                            gT_all[:, 2 * kk:2 * kk + 2,
                                   mt * 128:(mt + 1) * 128],
                            w2a[:, 2 * kk:2 * kk + 2,
                                nt2 * 384:(nt2 + 1) * 384],
                            start=(kk == 0), stop=(kk == FF // 256 - 1),
                            perf_mode=DR,
                        )
                    nc.vector.tensor_add(
                        out=acc[:, mt, nt2 * 384:(nt2 + 1) * 384],
                        in0=acc[:, mt, nt2 * 384:(nt2 + 1) * 384],
                        in1=ps[:])
                # ---- LN2 + store ----
                src_ = acc[:, mt, :]
                st = small.tile([128, 2, 6], F32, tag="lnst", bufs=8,
                                name="st2")
                for i in range(2):
                    nc.vector.bn_stats(out=st[:, i, :],
                                       in_=src_[:, i * 384:(i + 1) * 384])
                mv = small.tile([128, 2], F32, tag="lnmv", bufs=8, name="mv2")
                nc.vector.bn_aggr(out=mv[:], in_=st[:])
                sd = small.tile([128, 1], F32, tag="lnsd", bufs=8, name="sd2")
                nc.scalar.activation(sd[:], mv[:, 1:2],
                                     AF.Abs_reciprocal_sqrt, bias=eps_col[:])
                msd = small.tile([128, 1], F32, tag="lnms", bufs=8,
                                 name="msd2")
                nc.vector.tensor_scalar(
                    out=msd[:], in0=mv[:, 0:1], scalar1=sd[:],
                    scalar2=-1.0, op0=ALU.mult, op1=ALU.mult)
                ot = outp.tile([128, H], F32, tag="ot", name="ot")
                nc.scalar.activation(ot[:], src_, AF.Identity,
                                     scale=sd[:], bias=msd[:])
                # gamma on DVE, beta alternates gpsimd/DVE
                nc.vector.tensor_mul(ot[:], ot[:], l2g_bc[:])
                eng_b = nc.gpsimd if mt % 2 == 1 else nc.vector
                eng_b.tensor_add(ot[:], ot[:], l2b_bc[:])
                nc.sync.dma_start(
                    out=out_ext[mt * 128:(mt + 1) * 128, :], in_=ot)

    nc.finalize()
    return nc


_NC = None


def _get_nc():
    global _NC
    if _NC is None:
        _NC = build_nc()
    return _NC


def run(inputs, trace=False):
    f32 = lambda n: np.ascontiguousarray(np.asarray(inputs[n], dtype=np.float32))

    def bf16(a):
        return np.ascontiguousarray(a.astype(ml_dtypes.bfloat16))

    def fp8(a):
        return np.ascontiguousarray(
            np.clip(a, -448, 448).astype(ml_dtypes.float8_e4m3fn))

    hs = f32("hidden_state").reshape(NB, S, H)
    w1 = f32("W1")
    wo = f32("Wo")
    l1g = f32("ln1_g")
    l1b = f32("ln1_b")

    def bc128(v):
        return np.ascontiguousarray(np.broadcast_to(v, (128, H)).astype(np.float32))

    common = {
        "Wq": fp8(SQK * f32("Wq")), "bq": SQK * f32("bq"),
        "Wk": fp8(SQK * f32("Wk")), "bk": SQK * f32("bk"),
        "Wv": fp8(SQK * f32("Wv")),
        "Wo": fp8(SWO * wo),
        # fold the V bias through Wo:  softmax rows sum to 1
        "bo_eff_bc": bc128(SA * (f32("bo") + f32("bv") @ wo)),
        "ln1_g_bc": bc128(SW2 * l1g),
        "lb2_bc": bc128(SW2 * (l1b + f32("b2"))),
        # fold LN1 gamma/beta into the FFN input projection
        "W1g": fp8(SW1 * l1g[:, None] * w1),
        "b1f": np.ascontiguousarray(f32("b1") + l1b @ w1),
        "W2": fp8(SW2 * f32("W2")),
        "ln2_g_bc": bc128(f32("ln2_g")), "ln2_b_bc": bc128(f32("ln2_b")),
    }
    in_maps = []
    for i in range(NCORES):
        m = dict(common)
        x = np.ascontiguousarray(hs[i * BPC:(i + 1) * BPC].reshape(T, H))
        m["x_bf16"] = bf16(SA * x)
        m["xT"] = fp8(x.T)
        in_maps.append(m)
    res = run_bass_kernel_spmd(_get_nc(), in_maps, core_ids=list(range(NCORES)),
                               trace=trace)
    out = np.concatenate(
        [res.results[i]["out"].reshape(BPC, S, H) for i in range(NCORES)], axis=0)
    return out, res


def kernel(**inputs):
    return run(inputs)[0]


# revision 3
# speedup vs baseline: 1.2422x; 1.0712x over previous
"""BertLayer on 8 trn2 NeuronCores — data-parallel over batch (2 per core).

v10: full-fp8 (e4m3) with DoubleRow on the K>=256 contractions, restructured
for pipeline depth and p-state stability:
  - Scales folded host-side; LN1/LN2 scale-invariance absorbs them (x_nat at
    256x bf16, acc at 2048x f32); exp/gelu descale ride activation scales.
  - Startup: all four attention weight matrices + xT issued upfront across 3
    DMA queues; V runs kp-outer with 8 concurrent PSUM chains so the first
    matmul fires as soon as the first xT/Wv k-pair lands.
  - Attention: single-bank score tiles + per-head exp (scalar), 4-deep PSUM
    round-robin; ctx matmuls stay NORMAL-mode fp8 so the PE queue never
    drains (keeps the HAM clock at full rate); qk production is DoubleRow.
    Denominator staging (PSUM row 64 -> rows 0/32, recip, bf16 narrow) is
    entirely on DVE; scalar does only exps in steady state.
  - FFN: W1 fully resident; FFN1 runs nt=0 first, then FFN2 token blocks
    0..3 interleave with FFN1 nt=1 so the LN2/eviction chains drain during
    PE work and the post-matmul tail is one block deep.
  - W2 scaled 2048x so its fp8 encoding avoids the subnormal range.
"""

import sys

if "/opt/trn_rl_repo" not in sys.path:
    sys.path.insert(0, "/opt/trn_rl_repo")

from contextlib import ExitStack

import ml_dtypes
import numpy as np

import concourse.bass as bass
import concourse.tile as tile
from concourse import bacc, mybir
from concourse.masks import make_identity
from concourse.bass_utils import run_bass_kernel_spmd

F32 = mybir.dt.float32
BF16 = mybir.dt.bfloat16
FP8 = mybir.dt.float8e4
AF = mybir.ActivationFunctionType
ALU = mybir.AluOpType
DR = mybir.MatmulPerfMode.DoubleRow

# Problem dims (hardcoded: nn_BertLayer, hidden 768, 12 heads, ff 3072)
NB = 16
NCORES = 8
BPC = NB // NCORES
S = 512
T = BPC * S
H = 768
HK = H // 128
NH = 12
HD = 64
HP = HD + 4      # padded per-head stride in vA
FF = 3072
EPS = 1e-12
MT = T // 128
NQ = 4           # ffn chunks
FQ = FF // NQ    # 768 ff features per chunk
QK = FQ // 128   # 6 k-tiles per chunk
SCALE = 1.0 / float(np.sqrt(HD))

# fp8 scale plan (powers of two; LN scale-invariance absorbs them)
SQK = 64.0            # Wq*,Wk* = 64 W; qT,kT hold 64q, 64k
SV = 8.0              # vA holds 8 v   (Wv* = 64 Wv, evict scale 1/8)
SWO = 32.0            # Wo* = 32 Wo
SA = SV * SWO         # attn psum = 256 attn_out; x_nat = 256 x
SW1 = 64.0            # W1g* = 64 l1g W1; gelu activation scale 1/64
SW2 = 2048.0          # W2* = 2048 W2; acc carries 2048x
EXPSCALE = SCALE / (SQK * SQK)


def _col_ap(vec_ext, ntiles):
    a = vec_ext[:]
    return bass.AP(tensor=a.tensor, offset=a.offset, ap=[[1, 128], [128, ntiles]])


def build_nc():
    nc = bacc.Bacc(num_swdge_queues=4)

    xT_e = nc.declare_dram_parameter("xT", [H, T], FP8, isOutput=False)
    xn_e = nc.declare_dram_parameter("x_bf16", [T, H], BF16, isOutput=False)
    wq_e = nc.declare_dram_parameter("Wq", [H, H], FP8, isOutput=False)
    bq_e = nc.declare_dram_parameter("bq", [H], F32, isOutput=False)
    wk_e = nc.declare_dram_parameter("Wk", [H, H], FP8, isOutput=False)
    bk_e = nc.declare_dram_parameter("bk", [H], F32, isOutput=False)
    wv_e = nc.declare_dram_parameter("Wv", [H, H], FP8, isOutput=False)
    wo_e = nc.declare_dram_parameter("Wo", [H, H], FP8, isOutput=False)
    # pre-broadcast [128, H] vectors (contiguous DMA beats 128 descriptors)
    bo_b = nc.declare_dram_parameter("bo_eff_bc", [128, H], F32, isOutput=False)
    l1g_b = nc.declare_dram_parameter("ln1_g_bc", [128, H], F32, isOutput=False)
    l1b2_b = nc.declare_dram_parameter("lb2_bc", [128, H], F32, isOutput=False)
    w1_e = nc.declare_dram_parameter("W1g", [H, FF], FP8, isOutput=False)
    b1_e = nc.declare_dram_parameter("b1f", [FF], F32, isOutput=False)
    w2_e = nc.declare_dram_parameter("W2", [FF, H], FP8, isOutput=False)
    l2g_b = nc.declare_dram_parameter("ln2_g_bc", [128, H], F32, isOutput=False)
    l2b_b = nc.declare_dram_parameter("ln2_b_bc", [128, H], F32, isOutput=False)
    out_ext = nc.declare_dram_parameter("out", [T, H], F32, isOutput=True)

    with ExitStack() as top:
        tc = top.enter_context(tile.TileContext(nc))

        const = top.enter_context(tc.tile_pool(name="const", bufs=1))
        small = top.enter_context(tc.tile_pool(name="small", bufs=1))
        # two 4-deep pools of single-bank PSUM tiles
        ps_a = top.enter_context(tc.tile_pool(name="ps_a", bufs=4, space="PSUM"))
        ps_ctx = top.enter_context(tc.tile_pool(name="ps_ctx", bufs=4, space="PSUM"))
        main = top.enter_context(tc.tile_pool(name="main", bufs=1))
        wpool = top.enter_context(tc.tile_pool(name="wpool", bufs=4))

        eps_col = const.tile([128, 1], F32, name="eps_col")
        nc.vector.memset(eps_col, EPS)
        ones_all = const.tile([128, 64], BF16, name="ones_all")
        nc.vector.memset(ones_all, 1.0)
        ident = const.tile([128, 128], BF16, name="ident")
        make_identity(nc, ident)

        # -------- persistent tensors (slots recycled via tags) --------
        xT = main.tile([128, HK, T], FP8, tag="s1", name="xT")
        ctxT = main.tile([128, HK, T], FP8, tag="s2", name="ctxT")
        qT = main.tile([128, HK, T], FP8, tag="s3", bufs=1, name="qT")
        kT = main.tile([128, HK, T], FP8, tag="s4", name="kT")
        vA = main.tile([128, MT, NH, HP], FP8, tag="s5", name="vA")
        nc.vector.memset(vA[:, :, :, HD:HD + 1], 1.0)
        x_nat = main.tile([128, MT, H], BF16, tag="s6n", name="x_nat")

        # ------- input loads: EVERYTHING attention needs goes out up
        # front, interleaved across the three issuing engines -------
        wvsb = wpool.tile([128, HK, H], FP8, tag="wv", bufs=1, name="wvsb")
        wqsb = wpool.tile([128, HK, H], FP8, tag="wq", bufs=1, name="wqsb")
        wksb = wpool.tile([128, HK, H], FP8, tag="wk", bufs=1, name="wksb")
        # first V k-pair on all three queues, then round-robin the rest
        nc.sync.dma_start(out=xT[:, 0, :], in_=xT_e[0:128, :])
        nc.scalar.dma_start(out=wvsb[:, 0, :], in_=wv_e[0:128, :])
        nc.gpsimd.dma_start(out=xT[:, 1, :], in_=xT_e[128:256, :])
        nc.sync.dma_start(out=wvsb[:, 1, :], in_=wv_e[128:256, :])
        for kk in range(2, HK):
            nc.scalar.dma_start(
                out=xT[:, kk, :], in_=xT_e[kk * 128:(kk + 1) * 128, :])
            nc.gpsimd.dma_start(
                out=wvsb[:, kk, :], in_=wv_e[kk * 128:(kk + 1) * 128, :])
        for kk in range(HK):
            nc.sync.dma_start(
                out=wqsb[:, kk, :], in_=wq_e[kk * 128:(kk + 1) * 128, :])
            (nc.scalar if kk % 2 else nc.gpsimd).dma_start(
                out=wksb[:, kk, :], in_=wk_e[kk * 128:(kk + 1) * 128, :])

        # ---------------- V natural (dense PE warmup) ----------------
        # kp-outer with 8 concurrent chains: the first matmul only needs
        # the first xT/Wv k-pair, so compute overlaps the startup DMA.
        for half in range(2):
            vps = []
            for i in range(8):
                mt = half * 4 + i // 2
                nt2 = i % 2
                pool = ps_a if i < 4 else ps_ctx
                vps.append((mt, nt2, pool.tile(
                    [128, 384], F32, tag=("ps" if i < 4 else "ctx"),
                    name="psv")))
            for kp in range(HK // 2):
                for mt, nt2, ps in vps:
                    nc.tensor.matmul(
                        ps,
                        xT[:, 2 * kp:2 * kp + 2, mt * 128:(mt + 1) * 128],
                        wvsb[:, 2 * kp:2 * kp + 2, nt2 * 384:(nt2 + 1) * 384],
                        start=(kp == 0), stop=(kp == HK // 2 - 1),
                        perf_mode=DR,
                    )
            for mt, nt2, ps in vps:
                nc.scalar.activation(
                    vA[:, mt, nt2 * 6:(nt2 + 1) * 6, 0:HD],
                    ps.rearrange("p (h d) -> p h d", d=HD),
                    AF.Copy, scale=float(1.0 / SV),
                )

        # non-urgent loads (needed from the Wo phase on) go after the
        # attention-critical ones so they don't clog the DMA queues
        bq_cols = const.tile([128, HK], F32, name="bq_cols")
        nc.gpsimd.dma_start(out=bq_cols, in_=_col_ap(bq_e, HK))
        bk_cols = const.tile([128, HK], F32, name="bk_cols")
        nc.gpsimd.dma_start(out=bk_cols, in_=_col_ap(bk_e, HK))
        b1_cols = const.tile([128, FF // 128], F32, name="b1_cols")
        nc.gpsimd.dma_start(out=b1_cols, in_=_col_ap(b1_e, FF // 128))

        bo_bc = const.tile([128, H], F32, name="bo_bc")
        nc.gpsimd.dma_start(out=bo_bc, in_=bo_b[:, :])
        l1g_bc = const.tile([128, H], F32, name="l1g_bc")
        nc.gpsimd.dma_start(out=l1g_bc, in_=l1g_b[:, :])
        lb2_bc = const.tile([128, H], F32, name="lb2_bc")
        nc.gpsimd.dma_start(out=lb2_bc, in_=l1b2_b[:, :])
        l2g_bc = const.tile([128, H], F32, name="l2g_bc")
        nc.gpsimd.dma_start(out=l2g_bc, in_=l2g_b[:, :])
        l2b_bc = const.tile([128, H], F32, name="l2b_bc")
        nc.gpsimd.dma_start(out=l2b_bc, in_=l2b_b[:, :])
        for mt in range(MT):
            nc.gpsimd.dma_start(
                out=x_nat[:, mt, :], in_=xn_e[mt * 128:(mt + 1) * 128, :])
            nc.gpsimd.tensor_add(
                out=x_nat[:, mt, :], in0=x_nat[:, mt, :], in1=bo_bc[:])
        # Wo weights: needed ~40us in; issue now on the sync queue
        wosb = wpool.tile([128, HK, H], FP8, tag="wo", bufs=1, name="wosb")
        for kk in range(HK):
            nc.sync.dma_start(
                out=wosb[:, kk, :], in_=wo_e[kk * 128:(kk + 1) * 128, :])

        # ---------------- attention ----------------
        with ExitStack() as ph_ab:
            expp = ph_ab.enter_context(tc.tile_pool(name="expp", bufs=2))
            bcp = ph_ab.enter_context(tc.tile_pool(name="bcp", bufs=2))

            exp_tiles = {}
            ctx_ps = {}
            live = {}

            def qk_chain(t, i):
                """One of the 4 QK production chains for head-pair t."""
                wsb, b_cols, dstT = ((wqsb, bq_cols, qT),
                                     (wksb, bk_cols, kT))[i // 2]
                nt = i % 2
                ps = ps_a.tile([128, 512], F32, tag="ps", name="psqk")
                for kp in range(HK // 2):
                    nc.tensor.matmul(
                        ps,
                        wsb[:, 2 * kp:2 * kp + 2, t * 128:(t + 1) * 128],
                        xT[:, 2 * kp:2 * kp + 2, nt * 512:(nt + 1) * 512],
                        start=(kp == 0), stop=(kp == HK // 2 - 1),
                        perf_mode=DR,
                    )
                nc.vector.tensor_scalar_add(
                    out=dstT[:, t, nt * 512:(nt + 1) * 512],
                    in0=ps, scalar1=b_cols[:, t:t + 1],
                )

            def qk_pair(t):
                for i in range(4):
                    qk_chain(t, i)

            def s_single(t, b, kt, hh):
                """Scores for head 2t+hh, key block kt; one exp per head."""
                if kt == 0 and hh == 0:
                    exp_tiles[(t, b)] = expp.tile(
                        [128, 4, 2, 512], FP8, tag="expT", name="expT")
                expT = exp_tiles[(t, b)]
                ps2 = ps_a.tile([128, 512], F32, tag="ps", name="ps_s")
                poff = hh * 64
                nc.tensor.matmul(
                    ps2,
                    kT[poff:poff + 64, t,
                       b * 512 + kt * 128: b * 512 + (kt + 1) * 128],
                    qT[poff:poff + 64, t, b * 512:(b + 1) * 512],
                    start=True, stop=True,
                    tile_position=(poff, 0),
                )
                nc.scalar.activation(
                    expT[:, kt, hh, :], ps2, AF.Exp, scale=float(EXPSCALE))

            def ctx_chain(t, b, hh):
                """NORMAL-mode fp8 (keeps the PE queue full: clock stays up)."""
                expT = exp_tiles[(t, b)]
                h = 2 * t + hh
                ps_c = ps_ctx.tile([HD + 1, 512], F32, tag="ctx", name="ps_c")
                for kt in range(4):
                    nc.tensor.matmul(
                        ps_c,
                        vA[:, b * 4 + kt, h, 0:HD + 1],
                        expT[:, kt, hh, :],
                        start=(kt == 0), stop=(kt == 3),
                    )
                ctx_ps.setdefault((t, b), [None, None])[hh] = ps_c

            def den_chain(t, b):
                """Denominator rows -> SBUF rows 0/32 -> approx recip.
                All on DVE; scalar only does exps in steady state."""
                pcs = ctx_ps.pop((t, b))
                den2 = bcp.tile([128, 512], F32, tag="den", name="den2")
                nc.vector.tensor_copy(out=den2[0:1, :], in_=pcs[0][HD:HD + 1, :])
                nc.vector.tensor_copy(out=den2[32:33, :], in_=pcs[1][HD:HD + 1, :])
                rbf = bcp.tile([128, 512], F32, tag="rbf", name="rbf")
                nc.vector.reciprocal_approx_fast(
                    out=rbf[0:64, :], in_=den2[0:64, :])
                rb2 = bcp.tile([128, 512], BF16, tag="rb2", name="rb2")
                nc.vector.tensor_copy(out=rb2[0:33, :], in_=rbf[0:33, :])
                live[(t, b)] = (pcs, rb2)

            def nm_bcast(t, b):
                """Broadcast the reciprocals across partitions: both heads
                into ONE psum bank (head 1 lands on partitions 64..127 via
                col-group tiling), evicted with a single DVE copy."""
                pcs, rb2 = live[(t, b)]
                bc_sb = bcp.tile([128, 512], BF16, tag="bc", name="bc_sb")
                ps_b = ps_a.tile([128, 512], F32, tag="ps", name="ps_b")
                nc.tensor.matmul(
                    ps_b[0:64, :], ones_all[0:1, :],
                    rb2[0:1, :], start=True, stop=True)
                nc.tensor.matmul(
                    ps_b[64:128, :], ones_all[32:33, :],
                    rb2[32:33, :], start=True, stop=True)
                nc.vector.tensor_copy(out=bc_sb[:], in_=ps_b[:])
                live[(t, b)] = (pcs, bc_sb)

            def nm_mul(t, b):
                pcs, bc_sb = live.pop((t, b))
                for hh in range(2):
                    poff = hh * 64
                    nc.vector.tensor_mul(
                        out=ctxT[poff:poff + 64, t, b * 512:(b + 1) * 512],
                        in0=pcs[hh][0:64, :], in1=bc_sb[poff:poff + 64, :],
                    )

            # --- software pipeline: every exp is shadowed by dense MMs ---
            qk_pair(0)
            for t in range(HK):
                s_single(t, 0, 0, 0)
                s_single(t, 0, 0, 1)
                if t < HK - 1:
                    qk_chain(t + 1, 0)
                s_single(t, 0, 1, 0)
                s_single(t, 0, 1, 1)
                if t > 0:
                    nm_bcast(t - 1, 0)
                s_single(t, 0, 2, 0)
                s_single(t, 0, 2, 1)
                if t < HK - 1:
                    qk_chain(t + 1, 1)
                if t > 0:
                    nm_mul(t - 1, 0)
                s_single(t, 0, 3, 0)
                s_single(t, 0, 3, 1)
                if t > 0:
                    nm_bcast(t - 1, 1)
                s_single(t, 1, 0, 0)
                s_single(t, 1, 0, 1)
                if t < HK - 1:
                    qk_chain(t + 1, 2)
                if t > 0:
                    nm_mul(t - 1, 1)
                s_single(t, 1, 1, 0)
                s_single(t, 1, 1, 1)
                if t < HK - 1:
                    qk_chain(t + 1, 3)
                ctx_chain(t, 0, 0)
                s_single(t, 1, 2, 0)
                s_single(t, 1, 2, 1)
                ctx_chain(t, 0, 1)
                den_chain(t, 0)
                s_single(t, 1, 3, 0)
                s_single(t, 1, 3, 1)
                ctx_chain(t, 1, 0)
                ctx_chain(t, 1, 1)
                den_chain(t, 1)
            nm_bcast(HK - 1, 0)
            nm_mul(HK - 1, 0)

            # ------------- Wo + residual + LN1 + h transpose -------------
            # hT reuses xT's slot; acc reuses kT's.
            hT = main.tile([128, HK, T], FP8, tag="s1", name="hT")
            acc = main.tile([128, MT, H], F32, tag="s4", name="acc")
            with tc.tile_pool(name="attp", bufs=6) as attp:
                hbs = {}

                def emit_transposes(mt):
                    hb = hbs.pop(mt)
                    for c in range(0, HK, 2):
                        pt = ps_a.tile([128, 2, 128], BF16, tag="ps",
                                       name="pt")
                        for j in range(2):
                            nc.tensor.transpose(
                                pt[:, j, :],
                                hb[:, (c + j) * 128:(c + j + 1) * 128], ident)
                        nc.scalar.copy(
                            out=hT[:, c:c + 2, mt * 128:(mt + 1) * 128],
                            in_=pt[:])

                for mt in range(MT):
                    if mt == 1:
                        # rest of the attention epilogue, overlapped with
                        # the first Wo matmuls (they only need batch 0)
                        nm_bcast(HK - 1, 1)
                        nm_mul(HK - 1, 1)
                    # transposes lag two iterations so the PE never waits on
                    # the LN chain
                    if mt >= 2:
                        emit_transposes(mt - 2)
                    attn = attp.tile([128, H], F32, tag="attn", name="attn")
                    for nt2 in range(2):
                        ps = ps_ctx.tile([128, 384], F32, tag="ctx", name="psw")
                        for kp in range(HK // 2):
                            nc.tensor.matmul(
                                ps,
                                ctxT[:, 2 * kp:2 * kp + 2, mt * 128:(mt + 1) * 128],
                                wosb[:, 2 * kp:2 * kp + 2, nt2 * 384:(nt2 + 1) * 384],
                                start=(kp == 0), stop=(kp == HK // 2 - 1),
                                perf_mode=DR,
                            )
                        nc.vector.tensor_add(
                            out=attn[:, nt2 * 384:(nt2 + 1) * 384],
                            in0=ps[:], in1=x_nat[:, mt, nt2 * 384:(nt2 + 1) * 384])
                    # LN1 -> z in bf16 (gamma/beta folded into W1/b1); the
                    # normalize itself is ONE scalar-engine Identity activation
                    st = small.tile([128, 2, 6], F32, tag="lnst", bufs=8, name="st")
                    for i in range(2):
                        nc.vector.bn_stats(out=st[:, i, :],
                                           in_=attn[:, i * 384:(i + 1) * 384])
                    mv = small.tile([128, 2], F32, tag="lnmv", bufs=8, name="mv")
                    nc.vector.bn_aggr(out=mv[:], in_=st[:])
                    sd = small.tile([128, 1], F32, tag="lnsd", bufs=8, name="sd")
                    nc.scalar.activation(sd[:], mv[:, 1:2], AF.Abs_reciprocal_sqrt,
                                         bias=eps_col[:])
                    msd = small.tile([128, 1], F32, tag="lnms", bufs=8, name="msd")
                    nc.vector.tensor_scalar(
                        out=msd[:], in0=mv[:, 0:1], scalar1=sd[:], scalar2=-1.0,
                        op0=ALU.mult, op1=ALU.mult)
                    hb = attp.tile([128, H], BF16, tag="hb", name="hb")
                    nc.scalar.activation(hb[:], attn[:], AF.Identity,
                                         scale=sd[:], bias=msd[:])
                    hbs[mt] = hb
                    # residual path: acc = z*g1 + (ln1_b + b2), all 2048x
                    # (gpsimd, off the critical path)
                    nc.gpsimd.tensor_mul(acc[:, mt, :], hb[:], l1g_bc[:])
                    nc.gpsimd.tensor_add(acc[:, mt, :], acc[:, mt, :], lb2_bc[:])
                emit_transposes(MT - 2)
                emit_transposes(MT - 1)

        # ---------------- FFN ----------------
        # W1 fully resident; gT_all holds the WHOLE gelu output in fp8
        # (reuses vA's slot); W2 resident.  nt=0 features first, then FFN2
        # blocks 0..3 interleave with FFN1 nt=1 so LN2 chains drain under
        # PE work; tail is one block deep.
        gT_all = main.tile([128, FF // 128, T], FP8, tag="s5", name="gT_all")
        w1a = wpool.tile([128, HK, FF], FP8, tag="w1a", bufs=1, name="w1a")
        for q in range(NQ):
            for kk in range(HK):
                (nc.sync if kk % 2 else nc.scalar).dma_start(
                    out=w1a[:, kk, q * FQ:(q + 1) * FQ],
                    in_=w1_e[kk * 128:(kk + 1) * 128, q * FQ:(q + 1) * FQ])
        w2a = wpool.tile([128, FF // 128, H], FP8, tag="w2a", bufs=1,
                         name="w2a")
        for kk in range(FF // 128):
            eng = (nc.sync, nc.scalar, nc.gpsimd)[kk % 3]
            eng.dma_start(out=w2a[:, kk, :],
                          in_=w2_e[kk * 128:(kk + 1) * 128, :])
        with tc.tile_pool(name="outp", bufs=3) as outp:
            def ffn1_chunk(q, nt):
                for mo in range(QK):
                    ps = ps_ctx.tile([128, 512], F32, tag="ctx", name="psf1")
                    f = q * QK + mo
                    for kp in range(HK // 2):
                        nc.tensor.matmul(
                            ps,
                            w1a[:, 2 * kp:2 * kp + 2, f * 128:(f + 1) * 128],
                            hT[:, 2 * kp:2 * kp + 2, nt * 512:(nt + 1) * 512],
                            start=(kp == 0), stop=(kp == HK // 2 - 1),
                            perf_mode=DR,
                        )
                    nc.scalar.activation(
                        gT_all[:, f, nt * 512:(nt + 1) * 512], ps[:],
                        AF.Gelu, bias=b1_cols[:, f:f + 1],
                        scale=float(1.0 / SW1))

            def ffn2_block(mt):
                for nt2 in range(2):
                    ps = ps_a.tile([128, 384], F32, tag="ps", name="psf2")
                    for kk in range(FF // 256):
                        nc.tensor.matmul(
                            ps,
                            gT_all[:, 2 * kk:2 * kk + 2,
                                   mt * 128:(mt + 1) * 128],
                            w2a[:, 2 * kk:2 * kk + 2,
                                nt2 * 384:(nt2 + 1) * 384],
                            start=(kk == 0), stop=(kk == FF // 256 - 1),
                            perf_mode=DR,
                        )
                    nc.vector.tensor_add(
                        out=acc[:, mt, nt2 * 384:(nt2 + 1) * 384],
                        in0=acc[:, mt, nt2 * 384:(nt2 + 1) * 384],
                        in1=ps[:])
                # ---- LN2 + store ----
                src_ = acc[:, mt, :]
                st = small.tile([128, 2, 6], F32, tag="lnst", bufs=8,
                                name="st2")
                for i in range(2):
                    nc.vector.bn_stats(out=st[:, i, :],
                                       in_=src_[:, i * 384:(i + 1) * 384])
                mv = small.tile([128, 2], F32, tag="lnmv", bufs=8, name="mv2")
                nc.vector.bn_aggr(out=mv[:], in_=st[:])
                sd = small.tile([128, 1], F32, tag="lnsd", bufs=8, name="sd2")
                nc.scalar.activation(sd[:], mv[:, 1:2],
                                     AF.Abs_reciprocal_sqrt, bias=eps_col[:])
                msd = small.tile([128, 1], F32, tag="lnms", bufs=8,
                                 name="msd2")
                nc.vector.tensor_scalar(
                    out=msd[:], in0=mv[:, 0:1], scalar1=sd[:],
                    scalar2=-1.0, op0=ALU.mult, op1=ALU.mult)
                ot = outp.tile([128, H], F32, tag="ot", name="ot")
                nc.scalar.activation(ot[:], src_, AF.Identity,
                                     scale=sd[:], bias=msd[:])
                # gamma on DVE, beta alternates gpsimd/DVE
                nc.vector.tensor_mul(ot[:], ot[:], l2g_bc[:])
                eng_b = nc.gpsimd if mt % 2 == 1 else nc.vector
                eng_b.tensor_add(ot[:], ot[:], l2b_bc[:])
                nc.sync.dma_start(
                    out=out_ext[mt * 128:(mt + 1) * 128, :], in_=ot)

            for q in range(NQ):
                ffn1_chunk(q, 0)
            for q in range(NQ):
                ffn2_block(q)
                ffn1_chunk(q, 1)
            for mt in range(4, MT):
                ffn2_block(mt)

    nc.finalize()
    return nc


_NC = None


def _get_nc():
    global _NC
    if _NC is None:
        _NC = build_nc()
    return _NC


def run(inputs, trace=False):
    f32 = lambda n: np.ascontiguousarray(np.asarray(inputs[n], dtype=np.float32))

    def bf16(a):
        return np.ascontiguousarray(a.astype(ml_dtypes.bfloat16))

    def fp8(a):
        return np.ascontiguousarray(
            np.clip(a, -448, 448).astype(ml_dtypes.float8_e4m3fn))

    hs = f32("hidden_state").reshape(NB, S, H)
    w1 = f32("W1")
    wo = f32("Wo")
    l1g = f32("ln1_g")
    l1b = f32("ln1_b")

    def bc128(v):
        return np.ascontiguousarray(np.broadcast_to(v, (128, H)).astype(np.float32))

    common = {
        "Wq": fp8(SQK * f32("Wq")), "bq": SQK * f32("bq"),
        "Wk": fp8(SQK * f32("Wk")), "bk": SQK * f32("bk"),
        "Wv": fp8(SQK * f32("Wv")),
        "Wo": fp8(SWO * wo),
        # fold the V bias through Wo:  softmax rows sum to 1
        "bo_eff_bc": bc128(SA * (f32("bo") + f32("bv") @ wo)),
        "ln1_g_bc": bc128(SW2 * l1g),
        "lb2_bc": bc128(SW2 * (l1b + f32("b2"))),
        # fold LN1 gamma/beta into the FFN input projection
        "W1g": fp8(SW1 * l1g[:, None] * w1),
        "b1f": np.ascontiguousarray(f32("b1") + l1b @ w1),
        "W2": fp8(SW2 * f32("W2")),
        "ln2_g_bc": bc128(f32("ln2_g")), "ln2_b_bc": bc128(f32("ln2_b")),
    }
    in_maps = []
    for i in range(NCORES):
        m = dict(common)
        x = np.ascontiguousarray(hs[i * BPC:(i + 1) * BPC].reshape(T, H))
        m["x_bf16"] = bf16(SA * x)
        m["xT"] = fp8(x.T)
        in_maps.append(m)
    res = run_bass_kernel_spmd(_get_nc(), in_maps, core_ids=list(range(NCORES)),
                               trace=trace)
    out = np.concatenate(
        [res.results[i]["out"].reshape(BPC, S, H) for i in range(NCORES)], axis=0)
    return out, res


def kernel(**inputs):
    return run(inputs)[0]
